# revision 1
# baseline (speedup 1.0000x reference)
"""Trainium2 Bass kernel for nn_Net_53360673685530 (dehazing SGD loop).

Row-shard the [1017,1017] transmission map over 8 cores (128 own rows each +
64-row halos), keep all state in SBUF for the 100 iterations, exchange halos
via AllGather after iterations 32/64/96.  Per-core buffers are
[128 partitions, 2 chunks, 1019 cols] (row lr = 128*chunk + p, col guards at
j=0/1018).  Vertical stencils and raster-wrap column shifts run on the PE via
banded fp32 matmuls (bit-exact); transcendentals use only ln/exp (single ACT
table set, no swaps); elementwise work is split across DVE and GPSIMD; a
custom DVE op fuses (a-b)*s+c then square.
"""
import sys

for _p in ("/opt/trn_rl_repo", "/root/.axon_site/_ro/trn_rl_repo"):
    if _p not in sys.path:
        sys.path.insert(0, _p)

import numpy as np

import concourse.bass as bass
import concourse.tile as tile
from concourse import bacc, mybir, bass_utils, dve_ops
from concourse.dve_spec import Spec, Src0, Src1, sq, lower, _has_src1
from concourse.dve_spec import C0 as DC0, C1 as DC1
from concourse.dve_uop import DveOpSpec
from concourse.dve_ops import DveOp

FP = mybir.dt.float32
U32 = mybir.dt.uint32
AF = mybir.ActivationFunctionType
ALU = mybir.AluOpType

HP = WP = 1017
PATCH = 7
RATE = 0.001
C2R = 2.0 * RATE
N_ITERS = 100
NCORES = 8
OWN = 128
H = 64
NR = 256
F = WP + 2            # 1019
K_EXCH = 32
CONTRIB_ROWS = 3 * H  # 192: top-send, bottom-send, top-halo block
BIG_NEG = -1.0e30
LN48 = float(np.log(48.0))
FIX_ROWS_A = (0, 339, 678)
FIX_ROWS_B = (338, 677, 1016)

_NC_CACHE = {}
LAST_RESULTS = None


def _register_dve_op(name, spec):
    if name in dve_ops._SUB_OPCODE_FOR_NAME:
        return next(o for o in dve_ops.OPS if o.name == name)
    row = dve_ops._CUSTOM_DVE_ROW_BASE + len(dve_ops.OPS)
    assert row < 0x20
    shas = {}
    for ver in ("v3", "v4"):
        try:
            s = DveOpSpec(name=name, opcode=row, uops=lower(spec, ver=ver),
                          rd1_en=_has_src1(spec))
            shas[ver] = s.sha(ver)
        except Exception:
            pass
    op = DveOp(name, spec, subdim=False, uops_sha=shas)
    dve_ops.OPS.append(op)
    dve_ops._SUB_OPCODE_FOR_NAME[name] = row
    dve_ops.CUSTOM_DVE_SPECS[name] = spec
    return op


# out = ((in0 - in1) * s0 + s1)^2   (s0/s1: literal or [P,1] AP)
SQD = _register_dve_op("SQD_ANT", Spec(
    body=sq((Src0 - Src1) * DC0 + DC1),
    reference=lambda in0, in1, s0, s1, imm2:
        ((in0.astype(np.float32) - in1) * s0 + s1) ** 2,
))


# --------------------------- host-side helpers -----------------------------
def _host_sig(t_full, img, A):
    l = ((img[:HP, :WP, :] - A) / t_full[..., None] + A).astype(np.float32)
    flat = l.reshape(-1)
    Nf = HP * WP
    g = np.empty_like(flat)
    for r in range(3):
        seg = flat[r * Nf:(r + 1) * Nf]
        gs = np.empty_like(seg)
        gs[1:-1] = (seg[2:] - seg[:-2]) * np.float32(0.5)
        gs[0] = seg[1] - seg[0]
        gs[-1] = seg[-1] - seg[-2]
        g[r * Nf:(r + 1) * Nf] = gs
    y = g.reshape(Nf, 3)
    l2 = np.sqrt((y * y).sum(1, dtype=np.float32))
    with np.errstate(over="ignore"):
        sig = (1.0 / (1.0 + np.exp(np.float32(48.0) * (l2 - np.float32(0.1)))))
    return sig.reshape(HP, WP).astype(np.float32)


def _stencil_matrices():
    Z = lambda: np.zeros((128, 128), np.float32)
    Wdy = Z()
    for p in range(128):
        if p - 1 >= 0:
            Wdy[p - 1, p] = 1.0
        if p + 1 < 128:
            Wdy[p + 1, p] = -1.0
    Bdy01 = Z(); Bdy01[0, 127] = -1.0
    Bdy10 = Z(); Bdy10[127, 0] = 1.0
    Wgy = -Wdy
    Bgy01 = -Bdy01
    Bgy10 = -Bdy10
    Ieye = np.eye(128, dtype=np.float32)
    Sdn = Z()
    for p in range(1, 128):
        Sdn[p - 1, p] = 1.0   # out[p] = in[p-1]
    Sup = Z()
    for p in range(127):
        Sup[p + 1, p] = 1.0   # out[p] = in[p+1]
    return np.concatenate(
        [Wdy, Bdy01, Bdy10, Wgy, Bgy01, Bgy10, Ieye, Sdn, Sup], axis=1)


def _core_inputs(core, img, A, tlb, sig0, mats):
    start = OWN * core
    rowb = start - H
    glob = rowb + np.arange(NR)

    T0 = np.full((128, 2, F), 1.0, np.float32)
    Sc0 = np.zeros((128, 2, F), np.float32)
    N0 = np.zeros((128, 2, F), np.float32)
    N1 = np.zeros((128, 2, F), np.float32)
    N2 = np.zeros((128, 2, F), np.float32)
    for lr in range(NR):
        g = glob[lr]
        if 0 <= g < HP:
            c, p = lr // 128, lr % 128
            T0[p, c, 1:WP + 1] = tlb[g]
            Sc0[p, c, 1:WP + 1] = sig0[g]
            N0[p, c, 1:WP + 1] = img[g, :WP, 0] - A[0]
            N1[p, c, 1:WP + 1] = img[g, :WP, 1] - A[1]
            N2[p, c, 1:WP + 1] = img[g, :WP, 2] - A[2]

    biasRT = np.full((128, 2), np.log(C2R), np.float32)
    biasS = np.zeros((128, 2), np.float32)
    fixA = np.zeros((128, 2), np.float32)
    fixB = np.zeros((128, 2), np.float32)
    for lr in range(NR):
        c, p = lr // 128, lr % 128
        g = glob[lr]
        if not (0 <= g < HP):
            biasRT[p, c] += BIG_NEG
            biasS[p, c] += BIG_NEG
        if g in FIX_ROWS_A:
            fixA[p, c] = 1.0
        if g in FIX_ROWS_B:
            fixB[p, c] = 1.0
    A = A.astype(np.float32)
    c01 = np.float32(0.5) * (A[1] - A[2])
    c21 = np.float32(0.5) * (A[2] - A[0])
    c20 = np.float32(0.5) * (A[0] - A[1])
    consts = np.zeros((128, 10), np.float32)
    consts[:, 0] = c01
    consts[:, 1] = c21
    consts[:, 2] = c20
    consts[:, 3] = A[1] - A[0]   # fixup A: g0_alt const
    consts[:, 4] = A[2] - A[1]   # fixup B: g2_alt const
    consts[:, 5] = A[0]
    consts[:, 6] = A[1]
    consts[:, 7] = A[2]
    consts[:, 8] = LN48
    consts[:, 9] = -4.8
    bias16 = np.concatenate(
        [biasRT[:, 0:1], biasRT[:, 1:2], biasS[:, 0:1], biasS[:, 1:2],
         fixA[:, 0:1], fixA[:, 1:2], fixB[:, 0:1], fixB[:, 1:2], consts],
        axis=1)

    top_off = 2 * H if core == 0 else (core - 1) * CONTRIB_ROWS + H
    bot_off = 2 * H if core == NCORES - 1 else (core + 1) * CONTRIB_ROWS
    return {
        "t0_in": T0, "sc0_in": Sc0,
        "n0_in": N0, "n1_in": N1, "n2_in": N2,
        "mats_in": mats, "bias_in": bias16,
        "exoff_in": np.array([[top_off, bot_off]], np.uint32),
    }


# ------------------------------ kernel build -------------------------------
def _build(n_iters=N_ITERS, tap=None):
    key = (n_iters, tap)
    if key in _NC_CACHE:
        return _NC_CACHE[key]

    nc = bacc.Bacc("TRN2", target_bir_lowering=False, debug=False,
                   num_devices=NCORES)
    t0_in = nc.dram_tensor("t0_in", [128, 2, F], FP, kind="ExternalInput")
    sc0_in = nc.dram_tensor("sc0_in", [128, 2, F], FP, kind="ExternalInput")
    n0_in = nc.dram_tensor("n0_in", [128, 2, F], FP, kind="ExternalInput")
    n1_in = nc.dram_tensor("n1_in", [128, 2, F], FP, kind="ExternalInput")
    n2_in = nc.dram_tensor("n2_in", [128, 2, F], FP, kind="ExternalInput")
    mats_in = nc.dram_tensor("mats_in", [128, 9 * 128], FP, kind="ExternalInput")
    bias_in = nc.dram_tensor("bias_in", [128, 18], FP, kind="ExternalInput")
    exoff_in = nc.dram_tensor("exoff_in", [1, 2], U32, kind="ExternalInput")
    out_dram = nc.dram_tensor("out", [3, OWN, WP], FP, kind="ExternalOutput")

    IC = slice(1, F - 1)       # interior cols 1..1017
    JS = [(0, 512), (512, WP)]  # matmul free-dim slices

    with tile.TileContext(nc) as tc:
        with (
            tc.tile_pool(name="stat", bufs=1) as stat,
            tc.tile_pool(name="state", bufs=2) as state,
            tc.tile_pool(name="work", bufs=1) as work,
            tc.tile_pool(name="chain", bufs=2) as chain,
            tc.tile_pool(name="tiny", bufs=2) as tinyp,
            tc.tile_pool(name="psA", bufs=1, space=bass.MemorySpace.PSUM) as psA,
            tc.tile_pool(name="psB", bufs=1, space=bass.MemorySpace.PSUM) as psB,
            tc.tile_pool(name="dram", bufs=1, space="DRAM") as dram,
        ):
            MATS = stat.tile([128, 9 * 128], FP)
            BIAS = stat.tile([128, 18], FP)
            EXOFF = stat.tile([1, 2], U32)
            N0 = stat.tile([128, 2, F], FP)
            N1 = stat.tile([128, 2, F], FP)
            N2 = stat.tile([128, 2, F], FP)
            nc.sync.dma_start(MATS[:], mats_in[:])
            nc.sync.dma_start(BIAS[:], bias_in[:])
            nc.sync.dma_start(EXOFF[:], exoff_in[:])
            nc.sync.dma_start(N0[:], n0_in[:])
            nc.sync.dma_start(N1[:], n1_in[:])
            nc.sync.dma_start(N2[:], n2_in[:])

            Wdy = MATS[:, 0 * 128:1 * 128]
            Bdy01 = MATS[:, 1 * 128:2 * 128]
            Bdy10 = MATS[:, 2 * 128:3 * 128]
            Wgy = MATS[:, 3 * 128:4 * 128]
            Bgy01 = MATS[:, 4 * 128:5 * 128]
            Bgy10 = MATS[:, 5 * 128:6 * 128]
            Imat = MATS[:, 6 * 128:7 * 128]
            Sdn = MATS[:, 7 * 128:8 * 128]
            Sup = MATS[:, 8 * 128:9 * 128]
            bRT = [BIAS[:, 0:1], BIAS[:, 1:2]]
            bS = [BIAS[:, 2:3], BIAS[:, 3:4]]
            fA = [BIAS[:, 4:5], BIAS[:, 5:6]]
            fB = [BIAS[:, 6:7], BIAS[:, 7:8]]
            C01 = BIAS[:, 8:9]
            C21 = BIAS[:, 9:10]
            C20 = BIAS[:, 10:11]
            CA10 = BIAS[:, 11:12]
            CA21 = BIAS[:, 12:13]
            CA0 = BIAS[:, 13:14]
            CA1 = BIAS[:, 14:15]
            CA2 = BIAS[:, 15:16]
            CLN48 = BIAS[:, 16:17]
            CM48 = BIAS[:, 17:18]

            rtop = nc.alloc_registers("rtop", [mybir.EngineType.Pool])
            nc.regs_load(rtop, EXOFF[0:1, 0:1])
            top_off = nc.snap(rtop, donate=True, min_val=0,
                              max_val=NCORES * CONTRIB_ROWS - H)
            rbot = nc.alloc_registers("rbot", [mybir.EngineType.Pool])
            nc.regs_load(rbot, EXOFF[0:1, 1:2])
            bot_off = nc.snap(rbot, donate=True, min_val=0,
                              max_val=NCORES * CONTRIB_ROWS - H)

            contrib = dram.tile([CONTRIB_ROWS, WP], FP)
            gath = dram.tile([NCORES * CONTRIB_ROWS, WP], FP)

            # ---- initial state
            T = state.tile([128, 2, F], FP, tag="T")
            Sc = state.tile([128, 2, F], FP, tag="Sc")
            nc.sync.dma_start(T[:], t0_in[:])
            nc.sync.dma_start(Sc[:], sc0_in[:])
            L = state.tile([128, 2, F], FP, tag="L")
            nc.scalar.activation(L[:, :, :], T[:, :, :], AF.Ln)
            RTcm = state.tile([128, 2, F], FP, tag="RTcm")
            for c in range(2):
                nc.scalar.activation(RTcm[:, c, :], L[:, c, :], AF.Exp,
                                     bias=bRT[c], scale=-1.0)

            for it in range(1, n_iters + 1):
                # ================= A phase: T1 = T - (GX+GY)*RTcm ==========
                DX = work.tile([128, 2, F], FP, tag="w1")
                nc.vector.tensor_tensor(
                    DX[:, :, IC], L[:, :, 2:F], L[:, :, 0:F - 2], ALU.subtract)
                U = work.tile([128, 2, F], FP, tag="w2")
                nc.gpsimd.memset(U[:, :, 0:1], 0.0)
                nc.gpsimd.memset(U[:, :, F - 1:F], 0.0)
                nc.gpsimd.tensor_tensor(
                    U[:, :, IC], DX[:, :, IC], Sc[:, :, IC], ALU.mult)

                DYp = psA.tile([128, 2, 1024], FP, tag="psA")
                for c in range(2):
                    for j0, j1 in JS:
                        nc.tensor.matmul(DYp[:, c, j0:j1], Wdy,
                                         L[:, c, 1 + j0:1 + j1],
                                         start=True, stop=False)
                for j0, j1 in JS:
                    nc.tensor.matmul(DYp[:, 0, j0:j1], Bdy01,
                                     L[:, 1, 1 + j0:1 + j1],
                                     start=False, stop=True)
                for j0, j1 in JS:
                    nc.tensor.matmul(DYp[:, 1, j0:j1], Bdy10,
                                     L[:, 0, 1 + j0:1 + j1],
                                     start=False, stop=True)
                V = work.tile([128, 2, F], FP, tag="w3")
                nc.vector.tensor_tensor(
                    V[:, :, IC], DYp[:, :, 0:WP], Sc[:, :, IC], ALU.mult)

                GX = work.tile([128, 2, F], FP, tag="w4")
                nc.gpsimd.tensor_tensor(
                    GX[:, :, IC], U[:, :, 0:F - 2], U[:, :, 2:F], ALU.subtract)

                Gp = psB.tile([128, 2, 1024], FP, tag="psB")
                for c in range(2):
                    for j0, j1 in JS:
                        nc.tensor.matmul(Gp[:, c, j0:j1], Imat,
                                         GX[:, c, 1 + j0:1 + j1],
                                         start=True, stop=False)
                for c in range(2):
                    for j0, j1 in JS:
                        nc.tensor.matmul(Gp[:, c, j0:j1], Wgy,
                                         V[:, c, 1 + j0:1 + j1],
                                         start=False, stop=False)
                for j0, j1 in JS:
                    nc.tensor.matmul(Gp[:, 0, j0:j1], Bgy01,
                                     V[:, 1, 1 + j0:1 + j1],
                                     start=False, stop=True)
                for j0, j1 in JS:
                    nc.tensor.matmul(Gp[:, 1, j0:j1], Bgy10,
                                     V[:, 0, 1 + j0:1 + j1],
                                     start=False, stop=True)

                G2 = work.tile([128, 2, F], FP, tag="w5")
                nc.vector.tensor_tensor(
                    G2[:, :, IC], Gp[:, :, 0:WP], RTcm[:, :, IC], ALU.mult)
                T1 = state.tile([128, 2, F], FP, tag="T")
                nc.gpsimd.tensor_tensor(
                    T1[:, :, IC], T[:, :, IC], G2[:, :, IC], ALU.subtract)
                nc.gpsimd.memset(T1[:, :, 0:1], 1.0)
                nc.gpsimd.memset(T1[:, :, F - 1:F], 1.0)

                # ================= exchange ================================
                if it % K_EXCH == 0 and it < n_iters:
                    nc.gpsimd.dma_start(contrib[0:H, :], T1[H:128, 0, IC])
                    nc.gpsimd.dma_start(contrib[H:2 * H, :], T1[0:H, 1, IC])
                    nc.gpsimd.dma_start(contrib[2 * H:3 * H, :], T1[0:H, 0, IC])
                    nc.gpsimd.collective_compute(
                        "AllGather", ALU.bypass,
                        replica_groups=[list(range(NCORES))],
                        ins=[contrib.opt()], outs=[gath.opt()],
                    )
                    nc.gpsimd.dma_start(T1[0:H, 0, IC],
                                        gath[bass.ds(top_off, H), :])
                    nc.gpsimd.dma_start(T1[H:128, 1, IC],
                                        gath[bass.ds(bot_off, H), :])

                # ================= B phase: Sc1 from T1 ====================
                L1 = state.tile([128, 2, F], FP, tag="L")
                nc.scalar.activation(L1[:, :, :], T1[:, :, :], AF.Ln)
                RT1 = chain.tile([128, 2, F], FP, tag="rt")
                nc.scalar.activation(RT1[:, :, :], L1[:, :, :], AF.Exp,
                                     scale=-1.0)
                RTcm1 = state.tile([128, 2, F], FP, tag="RTcm")
                for c in range(2):
                    nc.scalar.activation(RTcm1[:, c, :], L1[:, c, :], AF.Exp,
                                         bias=bRT[c], scale=-1.0)

                P0 = work.tile([128, 2, F], FP, tag="w1")
                nc.gpsimd.tensor_tensor(P0[:, :, :], N0[:, :, :],
                                        RT1[:, :, :], ALU.mult)
                P1 = work.tile([128, 2, F], FP, tag="w2")
                nc.gpsimd.tensor_tensor(P1[:, :, :], N1[:, :, :],
                                        RT1[:, :, :], ALU.mult)
                P2 = work.tile([128, 2, F], FP, tag="w3")
                nc.vector.tensor_tensor(P2[:, :, :], N2[:, :, :],
                                        RT1[:, :, :], ALU.mult)

                # raster-wrap shifts via PE: P2SH[lr] = P2[lr-1, col 1017],
                # P0SH[lr] = P0[lr+1, col 1]
                SH = psA.tile([128, 2, 2], FP, tag="psA")
                for c in range(2):
                    nc.tensor.matmul(SH[:, c, 0:1], Sdn, P2[:, c, WP:WP + 1],
                                     start=True, stop=(c == 0))
                nc.tensor.matmul(SH[:, 1, 0:1], Bdy10, P2[:, 0, WP:WP + 1],
                                 start=False, stop=True)
                for c in range(2):
                    nc.tensor.matmul(SH[:, c, 1:2], Sup, P0[:, c, 1:2],
                                     start=True, stop=(c == 1))
                nc.tensor.matmul(SH[:, 0, 1:2], Bgy01, P0[:, 1, 1:2],
                                 start=False, stop=True)

                X0 = work.tile([128, 2, F], FP, tag="w4")
                X2 = work.tile([128, 2, F], FP, tag="w5")
                X1 = work.tile([128, 2, F], FP, tag="w6")
                for c in range(2):
                    nc.vector._custom_dve(
                        SQD, out=X0[:, c, 2:F - 1], in0=P1[:, c, 2:F - 1],
                        in1=P2[:, c, 1:F - 2], s0=0.5, s1=C01)
                    nc.vector._custom_dve(
                        SQD, out=X0[:, c, 1:2], in0=P1[:, c, 1:2],
                        in1=SH[:, c, 0:1], s0=0.5, s1=C01)
                    nc.vector._custom_dve(
                        SQD, out=X2[:, c, 1:F - 2], in0=P0[:, c, 2:F - 1],
                        in1=P1[:, c, 1:F - 2], s0=0.5, s1=C20)
                    nc.vector._custom_dve(
                        SQD, out=X2[:, c, WP:WP + 1], in0=SH[:, c, 1:2],
                        in1=P1[:, c, WP:WP + 1], s0=0.5, s1=C20)
                    nc.vector._custom_dve(
                        SQD, out=X1[:, c, IC], in0=P2[:, c, IC],
                        in1=P0[:, c, IC], s0=0.5, s1=C21)

                SSa = work.tile([128, 2, F], FP, tag="w7")
                nc.gpsimd.tensor_tensor(
                    SSa[:, :, IC], X0[:, :, IC], X1[:, :, IC], ALU.add)
                SS = work.tile([128, 2, F], FP, tag="w8")
                nc.vector.tensor_tensor(
                    SS[:, :, IC], SSa[:, :, IC], X2[:, :, IC], ALU.add)

                # fixups: 6 special flat-boundary pixels
                FAt = tinyp.tile([128, 2, 1], FP, tag="fa")
                FBt = tinyp.tile([128, 2, 1], FP, tag="fb")
                for c in range(2):
                    nc.vector._custom_dve(
                        SQD, out=FAt[:, c, 0:1], in0=P1[:, c, 1:2],
                        in1=P0[:, c, 1:2], s0=1.0, s1=CA10)
                    nc.vector._custom_dve(
                        SQD, out=FBt[:, c, 0:1], in0=P2[:, c, WP:WP + 1],
                        in1=P1[:, c, WP:WP + 1], s0=1.0, s1=CA21)
                TAt = tinyp.tile([128, 2, 1], FP, tag="ta")
                nc.vector.tensor_tensor(
                    TAt[:, :, 0:1], FAt[:, :, 0:1], X0[:, :, 1:2],
                    ALU.subtract)
                TBt = tinyp.tile([128, 2, 1], FP, tag="tb")
                nc.vector.tensor_tensor(
                    TBt[:, :, 0:1], FBt[:, :, 0:1], X2[:, :, WP:WP + 1],
                    ALU.subtract)
                for c in range(2):
                    nc.vector.scalar_tensor_tensor(
                        SS[:, c, 1:2], TAt[:, c, 0:1], fA[c], SS[:, c, 1:2],
                        ALU.mult, ALU.add)
                    nc.vector.scalar_tensor_tensor(
                        SS[:, c, WP:WP + 1], TBt[:, c, 0:1], fB[c],
                        SS[:, c, WP:WP + 1], ALU.mult, ALU.add)

                # sigma chain (ln/exp only):
                LSS = chain.tile([128, 2, F], FP, tag="ch")
                nc.scalar.activation(LSS[:, :, IC], SS[:, :, IC], AF.Ln)
                R48 = chain.tile([128, 2, F], FP, tag="ch")
                nc.scalar.activation(R48[:, :, IC], LSS[:, :, IC], AF.Exp,
                                     bias=CLN48, scale=0.5)
                E = chain.tile([128, 2, F], FP, tag="ch")
                nc.scalar.activation(E[:, :, IC], R48[:, :, IC], AF.Exp,
                                     bias=CM48, scale=1.0)
                A1p = chain.tile([128, 2, F], FP, tag="ch")
                nc.vector.tensor_scalar(A1p[:, :, IC], E[:, :, IC], 1.0e12,
                                        1.0, ALU.min, ALU.add)
                LA = chain.tile([128, 2, F], FP, tag="ch")
                nc.scalar.activation(LA[:, :, IC], A1p[:, :, IC], AF.Ln)
                Sc1 = state.tile([128, 2, F], FP, tag="Sc")
                for c in range(2):
                    nc.scalar.activation(Sc1[:, c, IC], LA[:, c, IC], AF.Exp,
                                         bias=bS[c], scale=-1.0)
                nc.gpsimd.memset(Sc1[:, :, 0:1], 0.0)
                nc.gpsimd.memset(Sc1[:, :, F - 1:F], 0.0)

                T, Sc, L, RTcm = T1, Sc1, L1, RTcm1
                if tap is not None and it == n_iters:
                    tapt = {"Sc": Sc1, "L": L1, "RTcm": RTcm1, "RT": RT1,
                            "SS": SS, "X0": X0, "X1": X1, "X2": X2,
                            "P0": P0, "P1": P1, "P2": P2, "E": E, "LA": LA,
                            "A1p": A1p, "LSS": LSS, "R48": R48}[tap]
                    for c in range(2):
                        nc.sync.dma_start(out_dram[c, :, :], tapt[:, c, IC])

            # ================= final output: N/T + A =======================
            if tap is None:
                RTf = work.tile([128, 2, F], FP, tag="w1")
                nc.vector.reciprocal(RTf[:, :, IC], T[:, :, IC])
                for ch, (Nt, Ac) in enumerate([(N0, CA0), (N1, CA1),
                                               (N2, CA2)]):
                    O = work.tile([128, 2, F], FP, tag="w2")
                    nc.vector.tensor_tensor(O[:, :, IC], Nt[:, :, IC],
                                            RTf[:, :, IC], ALU.mult)
                    nc.vector.tensor_scalar(O[:, :, IC], O[:, :, IC], Ac,
                                            None, ALU.add)
                    nc.sync.dma_start(out_dram[ch, 0:H, :], O[H:128, 0, IC])
                    nc.sync.dma_start(out_dram[ch, H:128, :], O[0:H, 1, IC])

    nc.compile()
    _NC_CACHE[key] = nc
    return nc


# ------------------------------- entry point -------------------------------
def kernel(img, airlight, patch_size):
    global LAST_RESULTS
    img = np.ascontiguousarray(np.asarray(img, dtype=np.float32))
    A = np.asarray(airlight, dtype=np.float32)
    p = int(patch_size)
    assert p == PATCH and img.shape == (1024, 1024, 3)

    center = img[p // 2:p // 2 + HP, p // 2:p // 2 + WP, :]
    tlb = np.max(1.0 - center / A, axis=-1).astype(np.float32)
    sig0 = _host_sig(tlb, img, A)

    mats = _stencil_matrices()
    in_maps = [_core_inputs(c, img, A, tlb, sig0, mats) for c in range(NCORES)]

    nc = _build(N_ITERS)
    res = bass_utils.run_bass_kernel_spmd(nc, in_maps,
                                          core_ids=list(range(NCORES)))
    LAST_RESULTS = res

    out = np.empty((HP, WP, 3), np.float32)
    for c in range(NCORES):
        o = res.results[c]["out"]          # [3, OWN, WP]
        nrows = min(OWN, HP - OWN * c)
        out[OWN * c:OWN * c + nrows, :, :] = o.transpose(1, 2, 0)[:nrows]
    return out


if __name__ == "__main__":
    d = np.load("/root/problem/ref_data.npz")
    out = kernel(d["img"], d["airlight"], 7)
    ref = np.load("/root/problem/ref_out.npy")
    err = np.abs(out - ref)
    print("max abs", err.max(), "l2rel",
          np.linalg.norm(out - ref) / np.linalg.norm(ref))



# revision 2
# speedup vs baseline: 6.6423x; 6.6423x over previous
"""Trainium2 Bass kernel for nn_Net_53360673685530 (dehazing SGD loop).

Row-shard the [1017,1017] transmission map over 8 cores (128 own rows each +
64-row halos), keep all state in SBUF for the 100 iterations, exchange halos
via AllGather after iterations 32/64/96.  Per-core buffers are
[128 partitions, 2 chunks, 1019 cols] (row lr = 128*chunk + p, col guards at
j=0/1018).  Vertical stencils and raster-wrap column shifts run on the PE via
banded fp32 matmuls (bit-exact); transcendentals use only ln/exp (single ACT
table set, no swaps); elementwise work is split across DVE and GPSIMD; a
custom DVE op fuses (a-b)*s+c then square.
"""
import sys

for _p in ("/opt/trn_rl_repo", "/root/.axon_site/_ro/trn_rl_repo"):
    if _p not in sys.path:
        sys.path.insert(0, _p)

import numpy as np

import concourse.bass as bass
import concourse.tile as tile
from concourse import bacc, mybir, bass_utils, dve_ops
from concourse.dve_spec import Spec, Src0, Src1, sq, lower, _has_src1
from concourse.dve_spec import C0 as DC0, C1 as DC1
from concourse.dve_uop import DveOpSpec
from concourse.dve_ops import DveOp

FP = mybir.dt.float32
U32 = mybir.dt.uint32
AF = mybir.ActivationFunctionType
ALU = mybir.AluOpType

# ---------------------------------------------------------------------------
# Steer the act-table placement pass: the kernel only uses Ln and Exp, and
# exactly one table set ("natural_log_exp_and_others") holds both.  The rust
# placement pass greedily picks the first set containing each function (ln ->
# "natural_log", exp -> "exp_and_others"), which makes every Ln<->Exp switch
# reload tables (1.3us each, ~5x per iteration).  Understate every other
# set's contents so the pass must pick the combined set for both functions;
# set indices are preserved, so the emitted act_func_set_id stays valid.
import concourse.hw_specs as _hw_specs
import concourse.bacc as _bacc_mod

_COMBINED_SET = "natural_log_exp_and_others"
_orig_get_tables = _hw_specs.get_activation_tables


def _patched_get_tables(arch):
    tabs = _orig_get_tables(arch)
    out = {}
    for name, s in tabs.items():
        if name == _COMBINED_SET:
            out[name] = set(s)
        else:
            out[name] = {f for f in s if f not in (AF.Ln, AF.Exp)}
    return out


_bacc_mod.get_activation_tables = _patched_get_tables

HP = WP = 1017
PATCH = 7
RATE = 0.001
C2R = 2.0 * RATE
N_ITERS = 100
NCORES = 8
OWN = 128
H = 64
NR = 256
F = WP + 2            # 1019
K_EXCH = 32
CONTRIB_ROWS = 3 * H  # 192: top-send, bottom-send, top-halo block
BIG_NEG = -1.0e30
LN48 = float(np.log(48.0))
FIX_ROWS_A = (0, 339, 678)
FIX_ROWS_B = (338, 677, 1016)

_NC_CACHE = {}
LAST_RESULTS = None


def _register_dve_op(name, spec):
    if name in dve_ops._SUB_OPCODE_FOR_NAME:
        return next(o for o in dve_ops.OPS if o.name == name)
    row = dve_ops._CUSTOM_DVE_ROW_BASE + len(dve_ops.OPS)
    assert row < 0x20
    shas = {}
    for ver in ("v3", "v4"):
        try:
            s = DveOpSpec(name=name, opcode=row, uops=lower(spec, ver=ver),
                          rd1_en=_has_src1(spec))
            shas[ver] = s.sha(ver)
        except Exception:
            pass
    op = DveOp(name, spec, subdim=False, uops_sha=shas)
    dve_ops.OPS.append(op)
    dve_ops._SUB_OPCODE_FOR_NAME[name] = row
    dve_ops.CUSTOM_DVE_SPECS[name] = spec
    return op


# out = ((in0 - in1) * s0 + s1)^2   (s0/s1: literal or [P,1] AP)
SQD = _register_dve_op("SQD_ANT", Spec(
    body=sq((Src0 - Src1) * DC0 + DC1),
    reference=lambda in0, in1, s0, s1, imm2:
        ((in0.astype(np.float32) - in1) * s0 + s1) ** 2,
))


# --------------------------- host-side helpers -----------------------------
def _host_sig(t_full, img, A):
    l = ((img[:HP, :WP, :] - A) / t_full[..., None] + A).astype(np.float32)
    flat = l.reshape(-1)
    Nf = HP * WP
    g = np.empty_like(flat)
    for r in range(3):
        seg = flat[r * Nf:(r + 1) * Nf]
        gs = np.empty_like(seg)
        gs[1:-1] = (seg[2:] - seg[:-2]) * np.float32(0.5)
        gs[0] = seg[1] - seg[0]
        gs[-1] = seg[-1] - seg[-2]
        g[r * Nf:(r + 1) * Nf] = gs
    y = g.reshape(Nf, 3)
    l2 = np.sqrt((y * y).sum(1, dtype=np.float32))
    with np.errstate(over="ignore"):
        sig = (1.0 / (1.0 + np.exp(np.float32(48.0) * (l2 - np.float32(0.1)))))
    return sig.reshape(HP, WP).astype(np.float32)


def _stencil_matrices():
    Z = lambda: np.zeros((128, 128), np.float32)
    Wdy = Z()
    for p in range(128):
        if p - 1 >= 0:
            Wdy[p - 1, p] = 1.0
        if p + 1 < 128:
            Wdy[p + 1, p] = -1.0
    Bdy01 = Z(); Bdy01[0, 127] = -1.0
    Bdy10 = Z(); Bdy10[127, 0] = 1.0
    Wgy = -Wdy
    Bgy01 = -Bdy01
    Bgy10 = -Bdy10
    Ieye = np.eye(128, dtype=np.float32)
    Sdn = Z()
    for p in range(1, 128):
        Sdn[p - 1, p] = 1.0   # out[p] = in[p-1]
    Sup = Z()
    for p in range(127):
        Sup[p + 1, p] = 1.0   # out[p] = in[p+1]
    return np.concatenate(
        [Wdy, Bdy01, Bdy10, Wgy, Bgy01, Bgy10, Ieye, Sdn, Sup], axis=1)


def _core_inputs(core, img, A, tlb, sig0, mats):
    start = OWN * core
    rowb = start - H
    glob = rowb + np.arange(NR)

    T0 = np.full((128, 2, F), 1.0, np.float32)
    Sc0 = np.zeros((128, 2, F), np.float32)
    N0 = np.zeros((128, 2, F), np.float32)
    N1 = np.zeros((128, 2, F), np.float32)
    N2 = np.zeros((128, 2, F), np.float32)
    for lr in range(NR):
        g = glob[lr]
        if 0 <= g < HP:
            c, p = lr // 128, lr % 128
            T0[p, c, 1:WP + 1] = tlb[g]
            Sc0[p, c, 1:WP + 1] = sig0[g]
            N0[p, c, 1:WP + 1] = img[g, :WP, 0] - A[0]
            N1[p, c, 1:WP + 1] = img[g, :WP, 1] - A[1]
            N2[p, c, 1:WP + 1] = img[g, :WP, 2] - A[2]

    biasRT = np.full((128, 2), np.log(C2R), np.float32)
    biasS = np.zeros((128, 2), np.float32)
    fixA = np.zeros((128, 2), np.float32)
    fixB = np.zeros((128, 2), np.float32)
    for lr in range(NR):
        c, p = lr // 128, lr % 128
        g = glob[lr]
        if not (0 <= g < HP):
            biasRT[p, c] += BIG_NEG
            biasS[p, c] += BIG_NEG
        if g in FIX_ROWS_A:
            fixA[p, c] = 1.0
        if g in FIX_ROWS_B:
            fixB[p, c] = 1.0
    A = A.astype(np.float32)
    c01 = np.float32(0.5) * (A[1] - A[2])
    c21 = np.float32(0.5) * (A[2] - A[0])
    c20 = np.float32(0.5) * (A[0] - A[1])
    consts = np.zeros((128, 10), np.float32)
    consts[:, 0] = c01
    consts[:, 1] = c21
    consts[:, 2] = c20
    consts[:, 3] = A[1] - A[0]   # fixup A: g0_alt const
    consts[:, 4] = A[2] - A[1]   # fixup B: g2_alt const
    consts[:, 5] = A[0]
    consts[:, 6] = A[1]
    consts[:, 7] = A[2]
    consts[:, 8] = LN48
    consts[:, 9] = -4.8
    bias16 = np.concatenate(
        [biasRT[:, 0:1], biasRT[:, 1:2], biasS[:, 0:1], biasS[:, 1:2],
         fixA[:, 0:1], fixA[:, 1:2], fixB[:, 0:1], fixB[:, 1:2], consts],
        axis=1)

    top_off = 2 * H if core == 0 else (core - 1) * CONTRIB_ROWS + H
    bot_off = 2 * H if core == NCORES - 1 else (core + 1) * CONTRIB_ROWS
    return {
        "t0_in": T0, "sc0_in": Sc0,
        "n0_in": N0, "n1_in": N1, "n2_in": N2,
        "mats_in": mats, "bias_in": bias16,
        "exoff_in": np.array([[top_off, bot_off]], np.uint32),
    }


# ------------------------------ kernel build -------------------------------
def _build(n_iters=N_ITERS, tap=None):
    key = (n_iters, tap)
    if key in _NC_CACHE:
        return _NC_CACHE[key]

    nc = bacc.Bacc("TRN2", target_bir_lowering=False, debug=False,
                   num_devices=NCORES)
    t0_in = nc.dram_tensor("t0_in", [128, 2, F], FP, kind="ExternalInput")
    sc0_in = nc.dram_tensor("sc0_in", [128, 2, F], FP, kind="ExternalInput")
    n0_in = nc.dram_tensor("n0_in", [128, 2, F], FP, kind="ExternalInput")
    n1_in = nc.dram_tensor("n1_in", [128, 2, F], FP, kind="ExternalInput")
    n2_in = nc.dram_tensor("n2_in", [128, 2, F], FP, kind="ExternalInput")
    mats_in = nc.dram_tensor("mats_in", [128, 9 * 128], FP, kind="ExternalInput")
    bias_in = nc.dram_tensor("bias_in", [128, 18], FP, kind="ExternalInput")
    exoff_in = nc.dram_tensor("exoff_in", [1, 2], U32, kind="ExternalInput")
    out_dram = nc.dram_tensor("out", [3, OWN, WP], FP, kind="ExternalOutput")

    IC = slice(1, F - 1)       # interior cols 1..1017
    JS = [(0, 512), (512, WP)]  # matmul free-dim slices

    with tile.TileContext(nc) as tc:
        with (
            tc.tile_pool(name="stat", bufs=1) as stat,
            tc.tile_pool(name="state", bufs=2) as state,
            tc.tile_pool(name="work", bufs=1) as work,
            tc.tile_pool(name="chain", bufs=2) as chain,
            tc.tile_pool(name="tiny", bufs=2) as tinyp,
            tc.tile_pool(name="psA", bufs=1, space=bass.MemorySpace.PSUM) as psA,
            tc.tile_pool(name="psB", bufs=1, space=bass.MemorySpace.PSUM) as psB,
            tc.tile_pool(name="dram", bufs=1, space="DRAM") as dram,
        ):
            MATS = stat.tile([128, 9 * 128], FP)
            BIAS = stat.tile([128, 18], FP)
            EXOFF = stat.tile([1, 2], U32)
            N0 = stat.tile([128, 2, F], FP)
            N1 = stat.tile([128, 2, F], FP)
            N2 = stat.tile([128, 2, F], FP)
            nc.sync.dma_start(MATS[:], mats_in[:])
            nc.sync.dma_start(BIAS[:], bias_in[:])
            nc.sync.dma_start(EXOFF[:], exoff_in[:])
            nc.sync.dma_start(N0[:], n0_in[:])
            nc.sync.dma_start(N1[:], n1_in[:])
            nc.sync.dma_start(N2[:], n2_in[:])

            Wdy = MATS[:, 0 * 128:1 * 128]
            Bdy01 = MATS[:, 1 * 128:2 * 128]
            Bdy10 = MATS[:, 2 * 128:3 * 128]
            Wgy = MATS[:, 3 * 128:4 * 128]
            Bgy01 = MATS[:, 4 * 128:5 * 128]
            Bgy10 = MATS[:, 5 * 128:6 * 128]
            Imat = MATS[:, 6 * 128:7 * 128]
            Sdn = MATS[:, 7 * 128:8 * 128]
            Sup = MATS[:, 8 * 128:9 * 128]
            bRT = [BIAS[:, 0:1], BIAS[:, 1:2]]
            bS = [BIAS[:, 2:3], BIAS[:, 3:4]]
            fA = [BIAS[:, 4:5], BIAS[:, 5:6]]
            fB = [BIAS[:, 6:7], BIAS[:, 7:8]]
            C01 = BIAS[:, 8:9]
            C21 = BIAS[:, 9:10]
            C20 = BIAS[:, 10:11]
            CA10 = BIAS[:, 11:12]
            CA21 = BIAS[:, 12:13]
            CA0 = BIAS[:, 13:14]
            CA1 = BIAS[:, 14:15]
            CA2 = BIAS[:, 15:16]
            CLN48 = BIAS[:, 16:17]
            CM48 = BIAS[:, 17:18]

            rtop = nc.alloc_registers("rtop", [mybir.EngineType.Pool])
            nc.regs_load(rtop, EXOFF[0:1, 0:1])
            top_off = nc.snap(rtop, donate=True, min_val=0,
                              max_val=NCORES * CONTRIB_ROWS - H)
            rbot = nc.alloc_registers("rbot", [mybir.EngineType.Pool])
            nc.regs_load(rbot, EXOFF[0:1, 1:2])
            bot_off = nc.snap(rbot, donate=True, min_val=0,
                              max_val=NCORES * CONTRIB_ROWS - H)

            contrib = dram.tile([CONTRIB_ROWS, WP], FP)
            gath = dram.tile([NCORES * CONTRIB_ROWS, WP], FP)

            # ---- initial state
            T = state.tile([128, 2, F], FP, tag="T")
            Sc = state.tile([128, 2, F], FP, tag="Sc")
            nc.sync.dma_start(T[:], t0_in[:])
            nc.sync.dma_start(Sc[:], sc0_in[:])
            L = state.tile([128, 2, F], FP, tag="L")
            nc.scalar.activation(L[:, :, :], T[:, :, :], AF.Ln)
            RTcm = state.tile([128, 2, F], FP, tag="RTcm")
            for c in range(2):
                nc.scalar.activation(RTcm[:, c, :], L[:, c, :], AF.Exp,
                                     bias=bRT[c], scale=-1.0)

            for it in range(1, n_iters + 1):
                # ================= A phase: T1 = T - (GX+GY)*RTcm ==========
                DX = work.tile([128, 2, F], FP, tag="w1")
                nc.vector.tensor_tensor(
                    DX[:, :, IC], L[:, :, 2:F], L[:, :, 0:F - 2], ALU.subtract)
                U = work.tile([128, 2, F], FP, tag="w2")
                nc.gpsimd.memset(U[:, :, 0:1], 0.0)
                nc.gpsimd.memset(U[:, :, F - 1:F], 0.0)
                nc.gpsimd.tensor_tensor(
                    U[:, :, IC], DX[:, :, IC], Sc[:, :, IC], ALU.mult)

                DYp = psA.tile([128, 2, 1024], FP, tag="psA")
                for c in range(2):
                    for j0, j1 in JS:
                        nc.tensor.matmul(DYp[:, c, j0:j1], Wdy,
                                         L[:, c, 1 + j0:1 + j1],
                                         start=True, stop=False)
                for j0, j1 in JS:
                    nc.tensor.matmul(DYp[:, 0, j0:j1], Bdy01,
                                     L[:, 1, 1 + j0:1 + j1],
                                     start=False, stop=True)
                for j0, j1 in JS:
                    nc.tensor.matmul(DYp[:, 1, j0:j1], Bdy10,
                                     L[:, 0, 1 + j0:1 + j1],
                                     start=False, stop=True)
                V = work.tile([128, 2, F], FP, tag="w3")
                nc.vector.tensor_tensor(
                    V[:, :, IC], DYp[:, :, 0:WP], Sc[:, :, IC], ALU.mult)

                GX = work.tile([128, 2, F], FP, tag="w4")
                nc.gpsimd.tensor_tensor(
                    GX[:, :, IC], U[:, :, 0:F - 2], U[:, :, 2:F], ALU.subtract)

                Gp = psB.tile([128, 2, 1024], FP, tag="psB")
                for c in range(2):
                    for j0, j1 in JS:
                        nc.tensor.matmul(Gp[:, c, j0:j1], Imat,
                                         GX[:, c, 1 + j0:1 + j1],
                                         start=True, stop=False)
                for c in range(2):
                    for j0, j1 in JS:
                        nc.tensor.matmul(Gp[:, c, j0:j1], Wgy,
                                         V[:, c, 1 + j0:1 + j1],
                                         start=False, stop=False)
                for j0, j1 in JS:
                    nc.tensor.matmul(Gp[:, 0, j0:j1], Bgy01,
                                     V[:, 1, 1 + j0:1 + j1],
                                     start=False, stop=True)
                for j0, j1 in JS:
                    nc.tensor.matmul(Gp[:, 1, j0:j1], Bgy10,
                                     V[:, 0, 1 + j0:1 + j1],
                                     start=False, stop=True)

                G2 = work.tile([128, 2, F], FP, tag="w5")
                nc.vector.tensor_tensor(
                    G2[:, :, IC], Gp[:, :, 0:WP], RTcm[:, :, IC], ALU.mult)
                T1 = state.tile([128, 2, F], FP, tag="T")
                nc.gpsimd.tensor_tensor(
                    T1[:, :, IC], T[:, :, IC], G2[:, :, IC], ALU.subtract)
                nc.gpsimd.memset(T1[:, :, 0:1], 1.0)
                nc.gpsimd.memset(T1[:, :, F - 1:F], 1.0)

                # ================= exchange ================================
                if it % K_EXCH == 0 and it < n_iters:
                    nc.gpsimd.dma_start(contrib[0:H, :], T1[H:128, 0, IC])
                    nc.gpsimd.dma_start(contrib[H:2 * H, :], T1[0:H, 1, IC])
                    nc.gpsimd.dma_start(contrib[2 * H:3 * H, :], T1[0:H, 0, IC])
                    nc.gpsimd.collective_compute(
                        "AllGather", ALU.bypass,
                        replica_groups=[list(range(NCORES))],
                        ins=[contrib.opt()], outs=[gath.opt()],
                    )
                    nc.gpsimd.dma_start(T1[0:H, 0, IC],
                                        gath[bass.ds(top_off, H), :])
                    nc.gpsimd.dma_start(T1[H:128, 1, IC],
                                        gath[bass.ds(bot_off, H), :])

                # ================= B phase: Sc1 from T1 ====================
                L1 = state.tile([128, 2, F], FP, tag="L")
                nc.scalar.activation(L1[:, :, :], T1[:, :, :], AF.Ln)
                RT1 = chain.tile([128, 2, F], FP, tag="rt")
                nc.scalar.activation(RT1[:, :, :], L1[:, :, :], AF.Exp,
                                     scale=-1.0)
                RTcm1 = state.tile([128, 2, F], FP, tag="RTcm")
                for c in range(2):
                    nc.scalar.activation(RTcm1[:, c, :], L1[:, c, :], AF.Exp,
                                         bias=bRT[c], scale=-1.0)

                P0 = work.tile([128, 2, F], FP, tag="w1")
                nc.gpsimd.tensor_tensor(P0[:, :, :], N0[:, :, :],
                                        RT1[:, :, :], ALU.mult)
                P1 = work.tile([128, 2, F], FP, tag="w2")
                nc.gpsimd.tensor_tensor(P1[:, :, :], N1[:, :, :],
                                        RT1[:, :, :], ALU.mult)
                P2 = work.tile([128, 2, F], FP, tag="w3")
                nc.vector.tensor_tensor(P2[:, :, :], N2[:, :, :],
                                        RT1[:, :, :], ALU.mult)

                # raster-wrap shifts via PE: P2SH[lr] = P2[lr-1, col 1017],
                # P0SH[lr] = P0[lr+1, col 1]
                SH = psA.tile([128, 2, 2], FP, tag="psA")
                for c in range(2):
                    nc.tensor.matmul(SH[:, c, 0:1], Sdn, P2[:, c, WP:WP + 1],
                                     start=True, stop=(c == 0))
                nc.tensor.matmul(SH[:, 1, 0:1], Bdy10, P2[:, 0, WP:WP + 1],
                                 start=False, stop=True)
                for c in range(2):
                    nc.tensor.matmul(SH[:, c, 1:2], Sup, P0[:, c, 1:2],
                                     start=True, stop=(c == 1))
                nc.tensor.matmul(SH[:, 0, 1:2], Bgy01, P0[:, 1, 1:2],
                                 start=False, stop=True)

                X0 = work.tile([128, 2, F], FP, tag="w4")
                X2 = work.tile([128, 2, F], FP, tag="w5")
                X1 = work.tile([128, 2, F], FP, tag="w6")
                for c in range(2):
                    nc.vector._custom_dve(
                        SQD, out=X0[:, c, 2:F - 1], in0=P1[:, c, 2:F - 1],
                        in1=P2[:, c, 1:F - 2], s0=0.5, s1=C01)
                    nc.vector._custom_dve(
                        SQD, out=X0[:, c, 1:2], in0=P1[:, c, 1:2],
                        in1=SH[:, c, 0:1], s0=0.5, s1=C01)
                    nc.vector._custom_dve(
                        SQD, out=X2[:, c, 1:F - 2], in0=P0[:, c, 2:F - 1],
                        in1=P1[:, c, 1:F - 2], s0=0.5, s1=C20)
                    nc.vector._custom_dve(
                        SQD, out=X2[:, c, WP:WP + 1], in0=SH[:, c, 1:2],
                        in1=P1[:, c, WP:WP + 1], s0=0.5, s1=C20)
                    nc.vector._custom_dve(
                        SQD, out=X1[:, c, IC], in0=P2[:, c, IC],
                        in1=P0[:, c, IC], s0=0.5, s1=C21)

                SSa = work.tile([128, 2, F], FP, tag="w7")
                nc.gpsimd.tensor_tensor(
                    SSa[:, :, IC], X0[:, :, IC], X1[:, :, IC], ALU.add)
                SS = work.tile([128, 2, F], FP, tag="w8")
                nc.vector.tensor_tensor(
                    SS[:, :, IC], SSa[:, :, IC], X2[:, :, IC], ALU.add)

                # fixups: 6 special flat-boundary pixels
                FAt = tinyp.tile([128, 2, 1], FP, tag="fa")
                FBt = tinyp.tile([128, 2, 1], FP, tag="fb")
                for c in range(2):
                    nc.vector._custom_dve(
                        SQD, out=FAt[:, c, 0:1], in0=P1[:, c, 1:2],
                        in1=P0[:, c, 1:2], s0=1.0, s1=CA10)
                    nc.vector._custom_dve(
                        SQD, out=FBt[:, c, 0:1], in0=P2[:, c, WP:WP + 1],
                        in1=P1[:, c, WP:WP + 1], s0=1.0, s1=CA21)
                TAt = tinyp.tile([128, 2, 1], FP, tag="ta")
                nc.vector.tensor_tensor(
                    TAt[:, :, 0:1], FAt[:, :, 0:1], X0[:, :, 1:2],
                    ALU.subtract)
                TBt = tinyp.tile([128, 2, 1], FP, tag="tb")
                nc.vector.tensor_tensor(
                    TBt[:, :, 0:1], FBt[:, :, 0:1], X2[:, :, WP:WP + 1],
                    ALU.subtract)
                for c in range(2):
                    nc.vector.scalar_tensor_tensor(
                        SS[:, c, 1:2], TAt[:, c, 0:1], fA[c], SS[:, c, 1:2],
                        ALU.mult, ALU.add)
                    nc.vector.scalar_tensor_tensor(
                        SS[:, c, WP:WP + 1], TBt[:, c, 0:1], fB[c],
                        SS[:, c, WP:WP + 1], ALU.mult, ALU.add)

                # sigma chain (ln/exp only):
                LSS = chain.tile([128, 2, F], FP, tag="ch")
                nc.scalar.activation(LSS[:, :, IC], SS[:, :, IC], AF.Ln)
                R48 = chain.tile([128, 2, F], FP, tag="ch")
                nc.scalar.activation(R48[:, :, IC], LSS[:, :, IC], AF.Exp,
                                     bias=CLN48, scale=0.5)
                E = chain.tile([128, 2, F], FP, tag="ch")
                nc.scalar.activation(E[:, :, IC], R48[:, :, IC], AF.Exp,
                                     bias=CM48, scale=1.0)
                A1p = chain.tile([128, 2, F], FP, tag="ch")
                nc.vector.tensor_scalar(A1p[:, :, IC], E[:, :, IC], 1.0e12,
                                        1.0, ALU.min, ALU.add)
                LA = chain.tile([128, 2, F], FP, tag="ch")
                nc.scalar.activation(LA[:, :, IC], A1p[:, :, IC], AF.Ln)
                Sc1 = state.tile([128, 2, F], FP, tag="Sc")
                for c in range(2):
                    nc.scalar.activation(Sc1[:, c, IC], LA[:, c, IC], AF.Exp,
                                         bias=bS[c], scale=-1.0)
                nc.gpsimd.memset(Sc1[:, :, 0:1], 0.0)
                nc.gpsimd.memset(Sc1[:, :, F - 1:F], 0.0)

                T, Sc, L, RTcm = T1, Sc1, L1, RTcm1
                if tap is not None and it == n_iters:
                    tapt = {"Sc": Sc1, "L": L1, "RTcm": RTcm1, "RT": RT1,
                            "SS": SS, "X0": X0, "X1": X1, "X2": X2,
                            "P0": P0, "P1": P1, "P2": P2, "E": E, "LA": LA,
                            "A1p": A1p, "LSS": LSS, "R48": R48}[tap]
                    for c in range(2):
                        nc.sync.dma_start(out_dram[c, :, :], tapt[:, c, IC])

            # ================= final output: N/T + A =======================
            if tap is None:
                RTf = work.tile([128, 2, F], FP, tag="w1")
                nc.vector.reciprocal(RTf[:, :, IC], T[:, :, IC])
                for ch, (Nt, Ac) in enumerate([(N0, CA0), (N1, CA1),
                                               (N2, CA2)]):
                    O = work.tile([128, 2, F], FP, tag="w2")
                    nc.vector.tensor_tensor(O[:, :, IC], Nt[:, :, IC],
                                            RTf[:, :, IC], ALU.mult)
                    nc.vector.tensor_scalar(O[:, :, IC], O[:, :, IC], Ac,
                                            None, ALU.add)
                    nc.sync.dma_start(out_dram[ch, 0:H, :], O[H:128, 0, IC])
                    nc.sync.dma_start(out_dram[ch, H:128, :], O[0:H, 1, IC])

    nc.compile()
    _NC_CACHE[key] = nc
    return nc


# ------------------------------- entry point -------------------------------
def kernel(img, airlight, patch_size):
    global LAST_RESULTS
    img = np.ascontiguousarray(np.asarray(img, dtype=np.float32))
    A = np.asarray(airlight, dtype=np.float32)
    p = int(patch_size)
    assert p == PATCH and img.shape == (1024, 1024, 3)

    center = img[p // 2:p // 2 + HP, p // 2:p // 2 + WP, :]
    tlb = np.max(1.0 - center / A, axis=-1).astype(np.float32)
    sig0 = _host_sig(tlb, img, A)

    mats = _stencil_matrices()
    in_maps = [_core_inputs(c, img, A, tlb, sig0, mats) for c in range(NCORES)]

    nc = _build(N_ITERS)
    res = bass_utils.run_bass_kernel_spmd(nc, in_maps,
                                          core_ids=list(range(NCORES)))
    LAST_RESULTS = res

    out = np.empty((HP, WP, 3), np.float32)
    for c in range(NCORES):
        o = res.results[c]["out"]          # [3, OWN, WP]
        nrows = min(OWN, HP - OWN * c)
        out[OWN * c:OWN * c + nrows, :, :] = o.transpose(1, 2, 0)[:nrows]
    return out


if __name__ == "__main__":
    d = np.load("/root/problem/ref_data.npz")
    out = kernel(d["img"], d["airlight"], 7)
    ref = np.load("/root/problem/ref_out.npy")
    err = np.abs(out - ref)
    print("max abs", err.max(), "l2rel",
          np.linalg.norm(out - ref) / np.linalg.norm(ref))

